# revision 7
# baseline (speedup 1.0000x reference)
"""Trainium2 Bass kernel for nn_Attention_45999099740384.

GQA attention over 8 independent packed sequences (block-diagonal mask with
equal blocks). Sharding: pure data-parallel over the 8 blocks — one block of
S=1024 tokens per NeuronCore, weights replicated, zero collectives (blocks
are fully independent; the output rows of block b depend only on x rows of
block b).

Per-core pipeline (bf16 TensorEngine, fp32 PSUM):
  1. natural-layout q/k/v projections from host-pre-transposed xT
  2. RoPE on VectorE (host-permuted wq/wk columns put the even/odd rotation
     pairs into contiguous 64-wide halves of each head)
  3. PE-transpose q/k into [head_dim, T] layout
  4. scores computed transposed: ST[s,q] = kT.T @ qT -> ScalarE exp ->
     P^T tiles in SBUF; P@V then needs NO transpose of P
     (out^T[d,q] = sum_s v[s,d] * PT[s,q], lhsT = v in natural layout)
  5. softmax row-sums via a ones[128,128] stationary matmul (replicates the
     sum across all partitions so the divide needs no partition broadcast);
     normalization deferred to after P@V
  6. wo matmul from the transposed attention output (already in the right
     layout), fp32 output.
"""

import numpy as np
import ml_dtypes

import concourse.bass as bass
import concourse.mybir as mybir
import concourse.tile as tile
from concourse import bacc
from concourse.bass_utils import run_bass_kernel_spmd
from concourse.masks import make_identity

# problem constants (hardcoded per task instructions)
DIM = 4096
N_HEADS = 32
HEAD_DIM = 128
N_KV = 8
REP = 4
B = 8
S = 1024
T = B * S

P = 128                  # SBUF partitions
KC = DIM // P            # 32 contraction chunks of 128
TT = S // P              # 8 token tiles per core
NCH = 512                # matmul moving free dim
SCALE = HEAD_DIM ** -0.5

F32 = mybir.dt.float32
BF16 = mybir.dt.bfloat16

_CACHE = {}


def build_nc():
    nc = bacc.Bacc("TRN2", target_bir_lowering=False, debug=False, num_devices=8)

    # per-core DRAM parameters (bf16 inputs prepared host-side)
    xt_d = nc.dram_tensor("xt", [P, KC, S], BF16, kind="ExternalInput")
    cs_d = nc.dram_tensor("cs", [P, TT, 64], F32, kind="ExternalInput")
    sn_d = nc.dram_tensor("sn", [P, TT, 64], F32, kind="ExternalInput")
    wq_d = nc.dram_tensor("wq", [KC, P, N_HEADS * HEAD_DIM], BF16, kind="ExternalInput")
    wk_d = nc.dram_tensor("wk", [KC, P, N_KV * HEAD_DIM], BF16, kind="ExternalInput")
    wv_d = nc.dram_tensor("wv", [KC, P, N_KV * HEAD_DIM], BF16, kind="ExternalInput")
    wo_d = nc.dram_tensor("wo", [KC, P, DIM], BF16, kind="ExternalInput")
    out_d = nc.dram_tensor("out", [S, DIM], F32, kind="ExternalOutput")
    # attention-output bounce (transposed layout for the wo matmul)
    otb_d = nc.dram_tensor("otb", [TT, P, N_HEADS, P], BF16)

    with tile.TileContext(nc) as tc:
        with (
            tc.tile_pool(name="const", bufs=1) as const,
            tc.tile_pool(name="wslab", bufs=2) as wslab_pool,
            tc.tile_pool(name="qtg", bufs=2) as qtg_pool,
            tc.tile_pool(name="nat", bufs=2) as nat_pool,
            tc.tile_pool(name="pt", bufs=2) as pt_pool,
            tc.tile_pool(name="scr", bufs=8) as scr_pool,
            tc.tile_pool(name="rc", bufs=2) as rc_pool,
            tc.tile_pool(name="otile", bufs=2) as ot_pool,
            tc.tile_pool(name="outp", bufs=3) as out_pool,
            tc.tile_pool(name="psmm", bufs=6, space="PSUM") as ps_pool,
            tc.tile_pool(name="pstp", bufs=2, space="PSUM") as tp_pool,
        ):
            # ---- constants ----
            ones_t = const.tile([P, P], BF16)
            nc.vector.memset(ones_t[:], 1.0)
            ident = const.tile([P, P], BF16)
            make_identity(nc, ident[:])

            cs = const.tile([P, TT, 64], F32)
            nc.sync.dma_start(out=cs[:], in_=cs_d.ap())
            sn = const.tile([P, TT, 64], F32)
            nc.sync.dma_start(out=sn[:], in_=sn_d.ap())

            kvres_cm = tc.tile_pool(name="kvres", bufs=1)
            kvres = kvres_cm.__enter__()
            kT = kvres.tile([P, N_KV, S], BF16)      # [d, kv, s]
            vN = kvres.tile([P, TT, N_KV * HEAD_DIM], BF16)  # [s_in, s_tile, kv*d]

            xres_cm = tc.tile_pool(name="xres", bufs=1)
            xres = xres_cm.__enter__()
            xt = xres.tile([P, KC, S], BF16)
            nc.sync.dma_start(out=xt[:], in_=xt_d.ap())

            def load_w_halves(w_dram, col0, tag):
                """Stream a [DIM, 512] weight column slab as 2 half-slabs."""
                halves = []
                for hf in range(2):
                    sl = wslab_pool.tile([P, KC // 2, NCH], BF16, tag=tag)
                    nc.sync.dma_start(
                        out=sl[:],
                        in_=w_dram.ap()[hf * (KC // 2):(hf + 1) * (KC // 2),
                                        :, col0:col0 + NCH]
                        .rearrange("kc p c -> p kc c"),
                    )
                    halves.append(sl)
                return halves

            def proj_psum(halves, tt):
                """psum[128 t, 512 cols] = x_tile @ W[:, cols]"""
                ps = ps_pool.tile([P, NCH], F32, tag="mm")
                for kc in range(KC):
                    nc.tensor.matmul(
                        ps[:],
                        lhsT=xt[:, kc, tt * P:(tt + 1) * P],
                        rhs=halves[kc // (KC // 2)][:, kc % (KC // 2), :],
                        start=(kc == 0),
                        stop=(kc == KC - 1),
                    )
                return ps

            def rope(ps, tt, nat):
                """RoPE on a [128 t, 4 heads x (64 even | 64 odd)] psum tile,
                writing bf16 into `nat` (same layout)."""
                v3 = ps[:].rearrange("p (h d) -> p h d", h=4)
                n3 = nat[:].rearrange("p (h d) -> p h d", h=4)
                qe = v3[:, :, 0:64]
                qo = v3[:, :, 64:128]
                cs_b = cs[:, tt, None, :].to_broadcast((P, 4, 64))
                sn_b = sn[:, tt, None, :].to_broadcast((P, 4, 64))
                s1 = scr_pool.tile([P, 4, 64], F32, tag="scr")
                s2 = scr_pool.tile([P, 4, 64], F32, tag="scr")
                nc.vector.tensor_tensor(s1[:], qe, cs_b, mybir.AluOpType.mult)
                nc.vector.tensor_tensor(s2[:], qo, sn_b, mybir.AluOpType.mult)
                nc.vector.tensor_tensor(n3[:, :, 0:64], s1[:], s2[:],
                                        mybir.AluOpType.subtract)
                s3 = scr_pool.tile([P, 4, 64], F32, tag="scr")
                s4 = scr_pool.tile([P, 4, 64], F32, tag="scr")
                nc.vector.tensor_tensor(s3[:], qe, sn_b, mybir.AluOpType.mult)
                nc.vector.tensor_tensor(s4[:], qo, cs_b, mybir.AluOpType.mult)
                nc.vector.tensor_tensor(n3[:, :, 64:128], s3[:], s4[:],
                                        mybir.AluOpType.add)

            def transpose_heads(nat, tt, dest, heads):
                """PE-transpose each [128 t, 128 d] head block of `nat` into
                dest[:, h, tt*128:(tt+1)*128] ([d, t] layout)."""
                for i, (h_slot) in enumerate(heads):
                    tp = tp_pool.tile([P, P], BF16, tag="tp")
                    nc.tensor.transpose(tp[:], nat[:, i * P:(i + 1) * P], ident[:])
                    nc.any.tensor_copy(
                        out=dest[:, h_slot, tt * P:(tt + 1) * P], in_=tp[:]
                    )

            # ---- k projection -> RoPE -> kT ----
            for cc in range(2):  # 4 kv heads per column chunk
                halves = load_w_halves(wk_d, cc * NCH, tag="wsl")
                for tt in range(TT):
                    ps = proj_psum(halves, tt)
                    nat = nat_pool.tile([P, NCH], BF16, tag="nat")
                    rope(ps, tt, nat)
                    transpose_heads(nat, tt, kT, [cc * 4 + j for j in range(4)])

            # ---- v projection (natural layout, no RoPE) ----
            for cc in range(2):
                halves = load_w_halves(wv_d, cc * NCH, tag="wsl")
                for tt in range(TT):
                    ps = proj_psum(halves, tt)
                    nc.any.tensor_copy(
                        out=vN[:, tt, cc * NCH:(cc + 1) * NCH], in_=ps[:]
                    )

            # ---- per kv-group: q projection + attention ----
            for g in range(N_KV):
                halves = load_w_halves(wq_d, g * NCH, tag="wsl")
                qT = qtg_pool.tile([P, REP, S], BF16, tag="qtg")  # [d, rep, t]
                for tt in range(TT):
                    ps = proj_psum(halves, tt)
                    nat = nat_pool.tile([P, NCH], BF16, tag="nat")
                    rope(ps, tt, nat)
                    transpose_heads(nat, tt, qT, list(range(REP)))

                for r in range(REP):
                    for qc in range(2):
                        pt = pt_pool.tile([P, TT, NCH], BF16, tag="pt")
                        for st in range(TT):
                            sps = ps_pool.tile([P, NCH], F32, tag="mm")
                            nc.tensor.matmul(
                                sps[:],
                                lhsT=kT[:, g, st * P:(st + 1) * P],
                                rhs=qT[:, r, qc * NCH:(qc + 1) * NCH],
                                start=True,
                                stop=True,
                            )
                            nc.scalar.activation(
                                pt[:, st, :], sps[:],
                                mybir.ActivationFunctionType.Exp,
                                scale=SCALE,
                            )
                        ops = ps_pool.tile([P, NCH], F32, tag="mm")
                        rps = ps_pool.tile([P, NCH], F32, tag="mm")
                        for st in range(TT):
                            nc.tensor.matmul(
                                ops[:],
                                lhsT=vN[:, st, g * P:(g + 1) * P],
                                rhs=pt[:, st, :],
                                start=(st == 0),
                                stop=(st == TT - 1),
                            )
                        for st in range(TT):
                            nc.tensor.matmul(
                                rps[:],
                                lhsT=ones_t[:],
                                rhs=pt[:, st, :],
                                start=(st == 0),
                                stop=(st == TT - 1),
                            )
                        rc = rc_pool.tile([P, NCH], F32, tag="rc")
                        nc.vector.reciprocal(rc[:], rps[:])
                        ot = ot_pool.tile([P, NCH], BF16, tag="ot")
                        nc.vector.tensor_tensor(ot[:], ops[:], rc[:],
                                                mybir.AluOpType.mult)
                        nc.sync.dma_start(
                            out=otb_d.ap()[qc * 4:(qc + 1) * 4, :, g * REP + r, :]
                            .rearrange("tt p t -> p tt t"),
                            in_=ot[:].rearrange("p (tt t) -> p tt t", tt=4),
                        )

            # ---- wo matmul ----
            # release x (and k/v) residency before allocating the
            # attention-output slab: their lifetimes don't overlap.
            xres_cm.__exit__(None, None, None)
            kvres_cm.__exit__(None, None, None)
            ores_cm = tc.tile_pool(name="ores", bufs=1)
            ores = ores_cm.__enter__()
            ot_all = ores.tile([P, TT, N_HEADS, P], BF16)
            nc.sync.dma_start(
                out=ot_all[:],
                in_=otb_d.ap().rearrange("tt p h t -> p tt h t"),
            )
            for nc5 in range(DIM // NCH):
                halves = load_w_halves(wo_d, nc5 * NCH, tag="wsl")
                for tt in range(TT):
                    ps = ps_pool.tile([P, NCH], F32, tag="mm")
                    for h in range(N_HEADS):
                        nc.tensor.matmul(
                            ps[:],
                            lhsT=ot_all[:, tt, h, :],
                            rhs=halves[h // (KC // 2)][:, h % (KC // 2), :],
                            start=(h == 0),
                            stop=(h == N_HEADS - 1),
                        )
                    outt = out_pool.tile([P, NCH], F32, tag="outp")
                    nc.vector.tensor_copy(out=outt[:], in_=ps[:])
                    nc.sync.dma_start(
                        out=out_d.ap()[tt * P:(tt + 1) * P,
                                       nc5 * NCH:(nc5 + 1) * NCH],
                        in_=outt[:],
                    )
            ores_cm.__exit__(None, None, None)

    nc.compile()
    return nc


# host-side input preparation -------------------------------------------------

_ROPE_PERM = np.concatenate([np.arange(0, HEAD_DIM, 2), np.arange(1, HEAD_DIM, 2)])


def _permute_heads(w, n_heads):
    """Permute columns within each head so rotation pairs become
    contiguous (even | odd) halves."""
    w = w.reshape(w.shape[0], n_heads, HEAD_DIM)
    return w[:, :, _ROPE_PERM].reshape(w.shape[0], n_heads * HEAD_DIM)


def _prep_shared(cos, sin, wq, wk, wv, wo):
    bf = ml_dtypes.bfloat16
    wq_p = _permute_heads(np.asarray(wq, dtype=np.float32), N_HEADS)
    wk_p = _permute_heads(np.asarray(wk, dtype=np.float32), N_KV)
    wq_l = np.ascontiguousarray(wq_p.reshape(KC, P, N_HEADS * HEAD_DIM)).astype(bf)
    wk_l = np.ascontiguousarray(wk_p.reshape(KC, P, N_KV * HEAD_DIM)).astype(bf)
    wv_l = np.ascontiguousarray(
        np.asarray(wv, dtype=np.float32).reshape(KC, P, N_KV * HEAD_DIM)
    ).astype(bf)
    wo_l = np.ascontiguousarray(
        np.asarray(wo, dtype=np.float32).reshape(KC, P, DIM)
    ).astype(bf)
    # positions restart at 0 per block, so block 0's table serves all cores
    cs_l = np.ascontiguousarray(
        np.asarray(cos[:S], dtype=np.float32).reshape(TT, P, 64).transpose(1, 0, 2)
    )
    sn_l = np.ascontiguousarray(
        np.asarray(sin[:S], dtype=np.float32).reshape(TT, P, 64).transpose(1, 0, 2)
    )
    return cs_l, sn_l, wq_l, wk_l, wv_l, wo_l


def _prep_x_block(xb):
    """x block [S, DIM] f32 -> xt [128, KC, S] bf16 (transposed)."""
    bf = ml_dtypes.bfloat16
    xtb = xb.T.reshape(KC, P, S).transpose(1, 0, 2)
    return np.ascontiguousarray(xtb).astype(bf)


def kernel(x, cos, sin, wq, wk, wv, wo):
    if "nc" not in _CACHE:
        _CACHE["nc"] = build_nc()
    nc = _CACHE["nc"]

    x = np.asarray(x, dtype=np.float32)
    cs_l, sn_l, wq_l, wk_l, wv_l, wo_l = _prep_shared(cos, sin, wq, wk, wv, wo)

    in_maps = []
    for b in range(B):
        in_maps.append({
            "xt": _prep_x_block(x[b * S:(b + 1) * S]),
            "cs": cs_l,
            "sn": sn_l,
            "wq": wq_l,
            "wk": wk_l,
            "wv": wv_l,
            "wo": wo_l,
        })
    res = run_bass_kernel_spmd(nc, in_maps, core_ids=list(range(B)))
    _CACHE["last_results"] = res
    out = np.concatenate([res.results[b]["out"] for b in range(B)], axis=0)
    return out.astype(np.float32)


# revision 11
# speedup vs baseline: 1.2741x; 1.2741x over previous
"""Trainium2 Bass kernel for nn_Attention_45999099740384.

GQA attention over 8 independent packed sequences (block-diagonal mask with
equal blocks). Sharding: pure data-parallel over the 8 blocks — one block of
S=1024 tokens per NeuronCore, weights replicated, zero collectives (blocks
are fully independent; the output rows of block b depend only on x rows of
block b).

Per-core pipeline (bf16 TensorEngine, fp32 PSUM):
  1. natural-layout q/k/v projections from host-pre-transposed xT
  2. RoPE on VectorE (host-permuted wq/wk columns put the even/odd rotation
     pairs into contiguous 64-wide halves of each head)
  3. PE-transpose q/k into [head_dim, T] layout
  4. scores computed transposed: ST[s,q] = kT.T @ qT -> ScalarE exp ->
     P^T tiles in SBUF; P@V then needs NO transpose of P
     (out^T[d,q] = sum_s v[s,d] * PT[s,q], lhsT = v in natural layout)
  5. softmax row-sums via a ones[128,128] stationary matmul; reciprocal on a
     single partition + GpSimd partition_broadcast; normalization deferred
     to after P@V
  6. wo matmul from the transposed attention output (already in the right
     layout), fp32 output.

Weights are shipped in a [n_col_chunks, 2, 128, 16, 512] layout so every
DMA slab is contiguous per partition, and large loads are split into
multiple dma_start instructions to spread across DMA queues.
"""

import numpy as np
import ml_dtypes

import concourse.bass as bass
import concourse.mybir as mybir
import concourse.tile as tile
from concourse import bacc
from concourse.bass_utils import run_bass_kernel_spmd
from concourse.masks import make_identity

# problem constants (hardcoded per task instructions)
DIM = 4096
N_HEADS = 32
HEAD_DIM = 128
N_KV = 8
REP = 4
B = 8
S = 1024
T = B * S

P = 128                  # SBUF partitions
KC = DIM // P            # 32 contraction chunks of 128
KH = KC // 2             # 16 chunks per half-slab
TT = S // P              # 8 token tiles per core
NCH = 512                # matmul moving free dim
SCALE = HEAD_DIM ** -0.5

F32 = mybir.dt.float32
BF16 = mybir.dt.bfloat16

_CACHE = {}


def build_nc():
    nc = bacc.Bacc("TRN2", target_bir_lowering=False, debug=False, num_devices=8)

    # per-core DRAM parameters (bf16, layouts prepared host-side)
    xt_d = nc.dram_tensor("xt", [P, KC, S], BF16, kind="ExternalInput")
    cs_d = nc.dram_tensor("cs", [P, TT, 64], F32, kind="ExternalInput")
    sn_d = nc.dram_tensor("sn", [P, TT, 64], F32, kind="ExternalInput")
    # weights: [n_col_chunks, 2 halves, 128 p, 16 kc, 512 c]
    wq_d = nc.dram_tensor("wq", [8, 2, P, KH, NCH], BF16, kind="ExternalInput")
    wk_d = nc.dram_tensor("wk", [2, 2, P, KH, NCH], BF16, kind="ExternalInput")
    wv_d = nc.dram_tensor("wv", [2, 2, P, KH, NCH], BF16, kind="ExternalInput")
    wo_d = nc.dram_tensor("wo", [8, 2, P, KH, NCH], BF16, kind="ExternalInput")
    out_d = nc.dram_tensor("out", [S, DIM], F32, kind="ExternalOutput")
    # attention-output bounce, [qc, d, h, t] so both sides are contiguous
    otb_d = nc.dram_tensor("otb", [2, P, N_HEADS, NCH], BF16)

    with tile.TileContext(nc) as tc:
        with (
            tc.tile_pool(name="const", bufs=1) as const,
            tc.tile_pool(name="wslab", bufs=2) as wslab_pool,
            tc.tile_pool(name="qtg", bufs=2) as qtg_pool,
            tc.tile_pool(name="nat", bufs=2) as nat_pool,
            tc.tile_pool(name="pt", bufs=2) as pt_pool,
            tc.tile_pool(name="scr", bufs=8) as scr_pool,
            tc.tile_pool(name="rc", bufs=2) as rc_pool,
            tc.tile_pool(name="bc", bufs=2) as bc_pool,
            tc.tile_pool(name="otile", bufs=2) as ot_pool,
            tc.tile_pool(name="outp", bufs=3) as out_pool,
            tc.tile_pool(name="psmm", bufs=4, space="PSUM") as ps_pool,
            tc.tile_pool(name="psst", bufs=1, space="PSUM") as st_pool,
            tc.tile_pool(name="pstp", bufs=2, space="PSUM") as tp_pool,
        ):
            # ---- constants ----
            ones_t = const.tile([P, P], BF16)
            nc.vector.memset(ones_t[:], 1.0)
            ident = const.tile([P, P], BF16)
            make_identity(nc, ident[:])

            cs = const.tile([P, TT, 64], F32)
            nc.sync.dma_start(out=cs[:], in_=cs_d.ap())
            sn = const.tile([P, TT, 64], F32)
            nc.sync.dma_start(out=sn[:], in_=sn_d.ap())

            kvres_cm = tc.tile_pool(name="kvres", bufs=1)
            kvres = kvres_cm.__enter__()
            kT = kvres.tile([P, N_KV, S], BF16)      # [d, kv, s]
            vN = kvres.tile([P, TT, N_KV * HEAD_DIM], BF16)  # [s_in, s_tile, kv*d]

            xres_cm = tc.tile_pool(name="xres", bufs=1)
            xres = xres_cm.__enter__()
            xt = xres.tile([P, KC, S], BF16)
            for i in range(4):
                nc.sync.dma_start(
                    out=xt[:, i * 8:(i + 1) * 8, :],
                    in_=xt_d.ap()[:, i * 8:(i + 1) * 8, :],
                )

            def load_w_halves(w_dram, cc, tag):
                """Stream one 512-col weight chunk as 2 contiguous half-slabs,
                each split into 2 dma_starts for queue parallelism."""
                halves = []
                for hf in range(2):
                    sl = wslab_pool.tile([P, KH, NCH], BF16, tag=tag)
                    for j in range(2):
                        nc.sync.dma_start(
                            out=sl[:, j * (KH // 2):(j + 1) * (KH // 2), :],
                            in_=w_dram.ap()[cc, hf, :,
                                            j * (KH // 2):(j + 1) * (KH // 2), :],
                        )
                    halves.append(sl)
                return halves

            def proj_psum(halves, tt):
                """psum[128 t, 512 cols] = x_tile @ W[:, cols]"""
                ps = ps_pool.tile([P, NCH], F32, tag="mm")
                for kc in range(KC):
                    nc.tensor.matmul(
                        ps[:],
                        lhsT=xt[:, kc, tt * P:(tt + 1) * P],
                        rhs=halves[kc // KH][:, kc % KH, :],
                        start=(kc == 0),
                        stop=(kc == KC - 1),
                    )
                return ps

            def rope(ps, tt, nat):
                """RoPE on a [128 t, 4 heads x (64 even | 64 odd)] psum tile,
                writing bf16 into `nat` (same layout)."""
                v3 = ps[:].rearrange("p (h d) -> p h d", h=4)
                n3 = nat[:].rearrange("p (h d) -> p h d", h=4)
                qe = v3[:, :, 0:64]
                qo = v3[:, :, 64:128]
                cs_b = cs[:, tt, None, :].to_broadcast((P, 4, 64))
                sn_b = sn[:, tt, None, :].to_broadcast((P, 4, 64))
                s1 = scr_pool.tile([P, 4, 64], F32, tag="scr")
                s2 = scr_pool.tile([P, 4, 64], F32, tag="scr")
                nc.vector.tensor_tensor(s1[:], qe, cs_b, mybir.AluOpType.mult)
                nc.vector.tensor_tensor(s2[:], qo, sn_b, mybir.AluOpType.mult)
                nc.vector.tensor_tensor(n3[:, :, 0:64], s1[:], s2[:],
                                        mybir.AluOpType.subtract)
                s3 = scr_pool.tile([P, 4, 64], F32, tag="scr")
                s4 = scr_pool.tile([P, 4, 64], F32, tag="scr")
                nc.vector.tensor_tensor(s3[:], qe, sn_b, mybir.AluOpType.mult)
                nc.vector.tensor_tensor(s4[:], qo, cs_b, mybir.AluOpType.mult)
                nc.vector.tensor_tensor(n3[:, :, 64:128], s3[:], s4[:],
                                        mybir.AluOpType.add)

            def transpose_heads(nat, tt, dest, h0):
                """PE-transpose the four [128 t, 128 d] head blocks of `nat`
                into dest[:, h0:h0+4, tt*128:(tt+1)*128] ([d, t] layout)."""
                tp = tp_pool.tile([P, 4, P], BF16, tag="tp")
                for i in range(4):
                    nc.tensor.transpose(tp[:, i, :], nat[:, i * P:(i + 1) * P],
                                        ident[:])
                nc.any.tensor_copy(
                    out=dest[:, h0:h0 + 4, tt * P:(tt + 1) * P], in_=tp[:]
                )

            # ---- k projection -> RoPE -> kT ----
            for cc in range(2):  # 4 kv heads per column chunk
                halves = load_w_halves(wk_d, cc, tag="wsl")
                for tt in range(TT):
                    ps = proj_psum(halves, tt)
                    nat = nat_pool.tile([P, NCH], BF16, tag="nat")
                    rope(ps, tt, nat)
                    transpose_heads(nat, tt, kT, cc * 4)

            # ---- v projection (natural layout, no RoPE) ----
            for cc in range(2):
                halves = load_w_halves(wv_d, cc, tag="wsl")
                for tt in range(TT):
                    ps = proj_psum(halves, tt)
                    nc.any.tensor_copy(
                        out=vN[:, tt, cc * NCH:(cc + 1) * NCH], in_=ps[:]
                    )

            # ---- per kv-group: q projection + attention ----
            for g in range(N_KV):
                halves = load_w_halves(wq_d, g, tag="wsl")
                qT = qtg_pool.tile([P, REP, S], BF16, tag="qtg")  # [d, rep, t]
                for tt in range(TT):
                    ps = proj_psum(halves, tt)
                    nat = nat_pool.tile([P, NCH], BF16, tag="nat")
                    rope(ps, tt, nat)
                    transpose_heads(nat, tt, qT, 0)

                for r in range(REP):
                    for qc in range(2):
                        pt = pt_pool.tile([P, TT, NCH], BF16, tag="pt")
                        for sp in range(TT // 2):  # st pairs share one exp
                            sps = st_pool.tile([P, 2 * NCH], F32, tag="st")
                            for j in range(2):
                                st = 2 * sp + j
                                nc.tensor.matmul(
                                    sps[:, j * NCH:(j + 1) * NCH],
                                    lhsT=kT[:, g, st * P:(st + 1) * P],
                                    rhs=qT[:, r, qc * NCH:(qc + 1) * NCH],
                                    start=True,
                                    stop=True,
                                )
                            nc.scalar.activation(
                                pt[:, 2 * sp:2 * sp + 2, :]
                                .rearrange("p a b -> p (a b)"),
                                sps[:],
                                mybir.ActivationFunctionType.Exp,
                                scale=SCALE,
                            )
                        ops = ps_pool.tile([P, NCH], F32, tag="mm")
                        rps = ps_pool.tile([P, NCH], F32, tag="mm")
                        for st in range(TT):
                            nc.tensor.matmul(
                                ops[:],
                                lhsT=vN[:, st, g * P:(g + 1) * P],
                                rhs=pt[:, st, :],
                                start=(st == 0),
                                stop=(st == TT - 1),
                            )
                        for st in range(TT):
                            nc.tensor.matmul(
                                rps[:],
                                lhsT=ones_t[:],
                                rhs=pt[:, st, :],
                                start=(st == 0),
                                stop=(st == TT - 1),
                            )
                        rc1 = rc_pool.tile([1, NCH], F32, tag="rc")
                        nc.vector.reciprocal(rc1[:], rps[0:1, :])
                        bc = bc_pool.tile([P, NCH], F32, tag="bc")
                        nc.gpsimd.partition_broadcast(bc[:], rc1[:])
                        ot = ot_pool.tile([P, NCH], BF16, tag="ot")
                        nc.vector.tensor_tensor(ot[:], ops[:], bc[:],
                                                mybir.AluOpType.mult)
                        nc.sync.dma_start(
                            out=otb_d.ap()[qc, :, g * REP + r, :],
                            in_=ot[:],
                        )

            # ---- wo matmul ----
            # release x (and k/v) residency before allocating the
            # attention-output slab: their lifetimes don't overlap.
            xres_cm.__exit__(None, None, None)
            kvres_cm.__exit__(None, None, None)
            ores_cm = tc.tile_pool(name="ores", bufs=1)
            ores = ores_cm.__enter__()
            ot_all = ores.tile([P, N_HEADS, S], BF16)  # [d, h, t]
            for qc in range(2):
                for j in range(4):
                    nc.sync.dma_start(
                        out=ot_all[:, j * 8:(j + 1) * 8, qc * NCH:(qc + 1) * NCH],
                        in_=otb_d.ap()[qc, :, j * 8:(j + 1) * 8, :],
                    )
            for nc5 in range(DIM // NCH):
                halves = load_w_halves(wo_d, nc5, tag="wsl")
                for tt in range(TT):
                    ps = ps_pool.tile([P, NCH], F32, tag="mm")
                    for h in range(N_HEADS):
                        nc.tensor.matmul(
                            ps[:],
                            lhsT=ot_all[:, h, tt * P:(tt + 1) * P],
                            rhs=halves[h // KH][:, h % KH, :],
                            start=(h == 0),
                            stop=(h == N_HEADS - 1),
                        )
                    outt = out_pool.tile([P, NCH], F32, tag="outp")
                    nc.vector.tensor_copy(out=outt[:], in_=ps[:])
                    nc.sync.dma_start(
                        out=out_d.ap()[tt * P:(tt + 1) * P,
                                       nc5 * NCH:(nc5 + 1) * NCH],
                        in_=outt[:],
                    )
            ores_cm.__exit__(None, None, None)

    nc.compile()
    return nc


# host-side input preparation -------------------------------------------------

_ROPE_PERM = np.concatenate([np.arange(0, HEAD_DIM, 2), np.arange(1, HEAD_DIM, 2)])


def _permute_heads(w, n_heads):
    """Permute columns within each head so rotation pairs become
    contiguous (even | odd) halves."""
    w = w.reshape(w.shape[0], n_heads, HEAD_DIM)
    return w[:, :, _ROPE_PERM].reshape(w.shape[0], n_heads * HEAD_DIM)


def _w_layout(w):
    """[DIM, C] f32 -> [C/512, 2, 128, 16, 512] bf16 slab layout."""
    C = w.shape[1]
    wl = w.reshape(2, KH, P, C // NCH, NCH).transpose(3, 0, 2, 1, 4)
    return np.ascontiguousarray(wl).astype(ml_dtypes.bfloat16)


def _prep_shared(cos, sin, wq, wk, wv, wo):
    wq_p = _permute_heads(np.asarray(wq, dtype=np.float32), N_HEADS)
    wk_p = _permute_heads(np.asarray(wk, dtype=np.float32), N_KV)
    wq_l = _w_layout(wq_p)
    wk_l = _w_layout(wk_p)
    wv_l = _w_layout(np.asarray(wv, dtype=np.float32))
    wo_l = _w_layout(np.asarray(wo, dtype=np.float32))
    # positions restart at 0 per block, so block 0's table serves all cores
    cs_l = np.ascontiguousarray(
        np.asarray(cos[:S], dtype=np.float32).reshape(TT, P, 64).transpose(1, 0, 2)
    )
    sn_l = np.ascontiguousarray(
        np.asarray(sin[:S], dtype=np.float32).reshape(TT, P, 64).transpose(1, 0, 2)
    )
    return cs_l, sn_l, wq_l, wk_l, wv_l, wo_l


def _prep_x_block(xb):
    """x block [S, DIM] f32 -> xt [128, KC, S] bf16 (transposed)."""
    bf = ml_dtypes.bfloat16
    xtb = xb.T.reshape(KC, P, S).transpose(1, 0, 2)
    return np.ascontiguousarray(xtb).astype(bf)


def kernel(x, cos, sin, wq, wk, wv, wo):
    if "nc" not in _CACHE:
        _CACHE["nc"] = build_nc()
    nc = _CACHE["nc"]

    x = np.asarray(x, dtype=np.float32)
    cs_l, sn_l, wq_l, wk_l, wv_l, wo_l = _prep_shared(cos, sin, wq, wk, wv, wo)

    in_maps = []
    for b in range(B):
        in_maps.append({
            "xt": _prep_x_block(x[b * S:(b + 1) * S]),
            "cs": cs_l,
            "sn": sn_l,
            "wq": wq_l,
            "wk": wk_l,
            "wv": wv_l,
            "wo": wo_l,
        })
    _CACHE["last_in_maps"] = in_maps
    res = run_bass_kernel_spmd(nc, in_maps, core_ids=list(range(B)))
    _CACHE["last_results"] = res
    out = np.concatenate([res.results[b]["out"] for b in range(B)], axis=0)
    return out.astype(np.float32)


# revision 12
# speedup vs baseline: 1.3435x; 1.0545x over previous
"""Trainium2 Bass kernel for nn_Attention_45999099740384.

GQA attention over 8 independent packed sequences (block-diagonal mask with
equal blocks). Sharding: pure data-parallel over the 8 blocks — one block of
S=1024 tokens per NeuronCore, weights replicated, zero collectives (blocks
are fully independent; the output rows of block b depend only on x rows of
block b).

Per-core pipeline (bf16 TensorEngine, fp32 PSUM):
  1. natural-layout q/k/v projections from host-pre-transposed xT
  2. RoPE on VectorE (host-permuted wq/wk columns put the even/odd rotation
     pairs into contiguous 64-wide halves of each head)
  3. PE-transpose q/k into [head_dim, T] layout
  4. scores computed transposed: ST[s,q] = kT.T @ qT -> ScalarE exp ->
     P^T tiles in SBUF; P@V then needs NO transpose of P
     (out^T[d,q] = sum_s v[s,d] * PT[s,q], lhsT = v in natural layout)
  5. softmax row-sums via a ones[128,128] stationary matmul; reciprocal on a
     single partition + GpSimd partition_broadcast; normalization deferred
     to after P@V
  6. wo matmul from the transposed attention output (already in the right
     layout), fp32 output.

Weights are shipped in a [n_col_chunks, 2, 128, 16, 512] layout so every
DMA slab is contiguous per partition, and large loads are split into
multiple dma_start instructions to spread across DMA queues.
"""

import numpy as np
import ml_dtypes

import concourse.bass as bass
import concourse.mybir as mybir
import concourse.tile as tile
from concourse import bacc
from concourse.bass_utils import run_bass_kernel_spmd
from concourse.masks import make_identity

# problem constants (hardcoded per task instructions)
DIM = 4096
N_HEADS = 32
HEAD_DIM = 128
N_KV = 8
REP = 4
B = 8
S = 1024
T = B * S

P = 128                  # SBUF partitions
KC = DIM // P            # 32 contraction chunks of 128
KH = KC // 2             # 16 chunks per half-slab
TT = S // P              # 8 token tiles per core
NCH = 512                # matmul moving free dim
SCALE = HEAD_DIM ** -0.5

F32 = mybir.dt.float32
BF16 = mybir.dt.bfloat16

_CACHE = {}


def build_nc():
    nc = bacc.Bacc("TRN2", target_bir_lowering=False, debug=False, num_devices=8)

    # per-core DRAM parameters (bf16, layouts prepared host-side)
    xt_d = nc.dram_tensor("xt", [P, KC, S], BF16, kind="ExternalInput")
    cs_d = nc.dram_tensor("cs", [P, TT, 64], F32, kind="ExternalInput")
    sn_d = nc.dram_tensor("sn", [P, TT, 64], F32, kind="ExternalInput")
    # weights: [n_col_chunks, 2 halves, 128 p, 16 kc, 512 c]
    wq_d = nc.dram_tensor("wq", [8, 2, P, KH, NCH], BF16, kind="ExternalInput")
    wk_d = nc.dram_tensor("wk", [2, 2, P, KH, NCH], BF16, kind="ExternalInput")
    wv_d = nc.dram_tensor("wv", [2, 2, P, KH, NCH], BF16, kind="ExternalInput")
    wo_d = nc.dram_tensor("wo", [8, 2, P, KH, NCH], BF16, kind="ExternalInput")
    out_d = nc.dram_tensor("out", [S, DIM], F32, kind="ExternalOutput")
    # attention-output bounce, [qc, d, h, t] so both sides are contiguous
    otb_d = nc.dram_tensor("otb", [2, P, N_HEADS, NCH], BF16)

    with tile.TileContext(nc) as tc:
        with (
            tc.tile_pool(name="const", bufs=1) as const,
            tc.tile_pool(name="wslab", bufs=3) as wslab_pool,
            tc.tile_pool(name="qtg", bufs=2) as qtg_pool,
            tc.tile_pool(name="nat", bufs=2) as nat_pool,
            tc.tile_pool(name="pt", bufs=2) as pt_pool,
            tc.tile_pool(name="scr", bufs=8) as scr_pool,
            tc.tile_pool(name="rcb", bufs=4) as rcb_pool,
            tc.tile_pool(name="otile", bufs=2) as ot_pool,
            tc.tile_pool(name="outp", bufs=3) as out_pool,
            tc.tile_pool(name="psmm", bufs=2, space="PSUM") as ps_pool,
            tc.tile_pool(name="psor", bufs=2, space="PSUM") as or_pool,
            tc.tile_pool(name="psst", bufs=2, space="PSUM") as st_pool,
            tc.tile_pool(name="pstp", bufs=2, space="PSUM") as tp_pool,
        ):
            # ---- constants ----
            ones_t = const.tile([P, P], BF16)
            nc.vector.memset(ones_t[:], 1.0)
            ident = const.tile([P, P], BF16)
            make_identity(nc, ident[:])

            cs = const.tile([P, TT, 64], F32)
            nc.sync.dma_start(out=cs[:], in_=cs_d.ap())
            sn = const.tile([P, TT, 64], F32)
            nc.sync.dma_start(out=sn[:], in_=sn_d.ap())

            kvres_cm = tc.tile_pool(name="kvres", bufs=1)
            kvres = kvres_cm.__enter__()
            kT = kvres.tile([P, N_KV, S], BF16)      # [d, kv, s]
            vN = kvres.tile([P, TT, N_KV * HEAD_DIM], BF16)  # [s_in, s_tile, kv*d]

            xres_cm = tc.tile_pool(name="xres", bufs=1)
            xres = xres_cm.__enter__()
            xt = xres.tile([P, KC, S], BF16)
            for i in range(4):
                nc.sync.dma_start(
                    out=xt[:, :, i * 256:(i + 1) * 256],
                    in_=xt_d.ap()[:, :, i * 256:(i + 1) * 256],
                )

            def load_w_halves(w_dram, cc, tag):
                """Stream one 512-col weight chunk as 2 contiguous half-slabs,
                each split into 2 dma_starts for queue parallelism."""
                halves = []
                for hf in range(2):
                    sl = wslab_pool.tile([P, KH, NCH], BF16, tag=tag)
                    for j in range(2):
                        nc.sync.dma_start(
                            out=sl[:, j * (KH // 2):(j + 1) * (KH // 2), :],
                            in_=w_dram.ap()[cc, hf, :,
                                            j * (KH // 2):(j + 1) * (KH // 2), :],
                        )
                    halves.append(sl)
                return halves

            def proj_psum(halves, tt):
                """psum[128 t, 512 cols] = x_tile @ W[:, cols]"""
                ps = ps_pool.tile([P, NCH], F32, tag="mm")
                for kc in range(KC):
                    nc.tensor.matmul(
                        ps[:],
                        lhsT=xt[:, kc, tt * P:(tt + 1) * P],
                        rhs=halves[kc // KH][:, kc % KH, :],
                        start=(kc == 0),
                        stop=(kc == KC - 1),
                    )
                return ps

            def rope(ps, tt, nat):
                """RoPE on a [128 t, 4 heads x (64 even | 64 odd)] psum tile,
                writing bf16 into `nat` (same layout)."""
                v3 = ps[:].rearrange("p (h d) -> p h d", h=4)
                n3 = nat[:].rearrange("p (h d) -> p h d", h=4)
                qe = v3[:, :, 0:64]
                qo = v3[:, :, 64:128]
                cs_b = cs[:, tt, None, :].to_broadcast((P, 4, 64))
                sn_b = sn[:, tt, None, :].to_broadcast((P, 4, 64))
                s1 = scr_pool.tile([P, 4, 64], F32, tag="scr")
                s2 = scr_pool.tile([P, 4, 64], F32, tag="scr")
                nc.vector.tensor_tensor(s1[:], qe, cs_b, mybir.AluOpType.mult)
                nc.vector.tensor_tensor(s2[:], qo, sn_b, mybir.AluOpType.mult)
                nc.vector.tensor_tensor(n3[:, :, 0:64], s1[:], s2[:],
                                        mybir.AluOpType.subtract)
                s3 = scr_pool.tile([P, 4, 64], F32, tag="scr")
                s4 = scr_pool.tile([P, 4, 64], F32, tag="scr")
                nc.vector.tensor_tensor(s3[:], qe, sn_b, mybir.AluOpType.mult)
                nc.vector.tensor_tensor(s4[:], qo, cs_b, mybir.AluOpType.mult)
                nc.vector.tensor_tensor(n3[:, :, 64:128], s3[:], s4[:],
                                        mybir.AluOpType.add)

            def transpose_heads(nat, tt, dest, h0):
                """PE-transpose the four [128 t, 128 d] head blocks of `nat`
                into dest[:, h0:h0+4, tt*128:(tt+1)*128] ([d, t] layout)."""
                tp = tp_pool.tile([P, 4, P], BF16, tag="tp")
                for i in range(4):
                    nc.tensor.transpose(tp[:, i, :], nat[:, i * P:(i + 1) * P],
                                        ident[:])
                nc.any.tensor_copy(
                    out=dest[:, h0:h0 + 4, tt * P:(tt + 1) * P], in_=tp[:]
                )

            # ---- k projection -> RoPE -> kT ----
            for cc in range(2):  # 4 kv heads per column chunk
                halves = load_w_halves(wk_d, cc, tag="wsl")
                for tt in range(TT):
                    ps = proj_psum(halves, tt)
                    nat = nat_pool.tile([P, NCH], BF16, tag="nat")
                    rope(ps, tt, nat)
                    transpose_heads(nat, tt, kT, cc * 4)

            # ---- v projection (natural layout, no RoPE) ----
            for cc in range(2):
                halves = load_w_halves(wv_d, cc, tag="wsl")
                for tt in range(TT):
                    ps = proj_psum(halves, tt)
                    nc.any.tensor_copy(
                        out=vN[:, tt, cc * NCH:(cc + 1) * NCH], in_=ps[:]
                    )

            # ---- per kv-group: q projection + attention ----
            for g in range(N_KV):
                halves = load_w_halves(wq_d, g, tag="wsl")
                qT = qtg_pool.tile([P, REP, S], BF16, tag="qtg")  # [d, rep, t]
                for tt in range(TT):
                    ps = proj_psum(halves, tt)
                    nat = nat_pool.tile([P, NCH], BF16, tag="nat")
                    rope(ps, tt, nat)
                    transpose_heads(nat, tt, qT, 0)

                for r in range(REP):
                    for qc in range(2):
                        pt = pt_pool.tile([P, TT, NCH], BF16, tag="pt")
                        for st in range(TT):
                            sps = st_pool.tile([P, NCH], F32, tag="st")
                            nc.tensor.matmul(
                                sps[:],
                                lhsT=kT[:, g, st * P:(st + 1) * P],
                                rhs=qT[:, r, qc * NCH:(qc + 1) * NCH],
                                start=True,
                                stop=True,
                            )
                            nc.scalar.activation(
                                pt[:, st, :], sps[:],
                                mybir.ActivationFunctionType.Exp,
                                scale=SCALE,
                            )
                        ops = or_pool.tile([P, NCH], F32, tag="ors")
                        rps = or_pool.tile([P, NCH], F32, tag="ors")
                        for st in range(TT):
                            nc.tensor.matmul(
                                ops[:],
                                lhsT=vN[:, st, g * P:(g + 1) * P],
                                rhs=pt[:, st, :],
                                start=(st == 0),
                                stop=(st == TT - 1),
                            )
                        for st in range(TT):
                            nc.tensor.matmul(
                                rps[:],
                                lhsT=ones_t[:],
                                rhs=pt[:, st, :],
                                start=(st == 0),
                                stop=(st == TT - 1),
                            )
                        lnt = rcb_pool.tile([P, NCH], F32, tag="rcb")
                        nc.scalar.activation(lnt[:], rps[:],
                                             mybir.ActivationFunctionType.Ln)
                        rcb = rcb_pool.tile([P, NCH], F32, tag="rcb")
                        nc.scalar.activation(rcb[:], lnt[:],
                                             mybir.ActivationFunctionType.Exp,
                                             scale=-1.0)
                        ot = ot_pool.tile([P, NCH], BF16, tag="ot")
                        nc.vector.tensor_tensor(ot[:], ops[:], rcb[:],
                                                mybir.AluOpType.mult)
                        nc.sync.dma_start(
                            out=otb_d.ap()[qc, :, g * REP + r, :],
                            in_=ot[:],
                        )

            # ---- wo matmul ----
            # release x (and k/v) residency before allocating the
            # attention-output slab: their lifetimes don't overlap.
            xres_cm.__exit__(None, None, None)
            kvres_cm.__exit__(None, None, None)
            ores_cm = tc.tile_pool(name="ores", bufs=1)
            ores = ores_cm.__enter__()
            ot_all = ores.tile([P, N_HEADS, S], BF16)  # [d, h, t]
            for qc in range(2):
                for j in range(4):
                    nc.sync.dma_start(
                        out=ot_all[:, j * 8:(j + 1) * 8, qc * NCH:(qc + 1) * NCH],
                        in_=otb_d.ap()[qc, :, j * 8:(j + 1) * 8, :],
                    )
            for nc5 in range(DIM // NCH):
                halves = load_w_halves(wo_d, nc5, tag="wsl")
                for tt in range(TT):
                    ps = or_pool.tile([P, NCH], F32, tag="ors")
                    for h in range(N_HEADS):
                        nc.tensor.matmul(
                            ps[:],
                            lhsT=ot_all[:, h, tt * P:(tt + 1) * P],
                            rhs=halves[h // KH][:, h % KH, :],
                            start=(h == 0),
                            stop=(h == N_HEADS - 1),
                        )
                    outt = out_pool.tile([P, NCH], F32, tag="outp")
                    nc.vector.tensor_copy(out=outt[:], in_=ps[:])
                    nc.sync.dma_start(
                        out=out_d.ap()[tt * P:(tt + 1) * P,
                                       nc5 * NCH:(nc5 + 1) * NCH],
                        in_=outt[:],
                    )
            ores_cm.__exit__(None, None, None)

    nc.compile()
    return nc


# host-side input preparation -------------------------------------------------

_ROPE_PERM = np.concatenate([np.arange(0, HEAD_DIM, 2), np.arange(1, HEAD_DIM, 2)])


def _permute_heads(w, n_heads):
    """Permute columns within each head so rotation pairs become
    contiguous (even | odd) halves."""
    w = w.reshape(w.shape[0], n_heads, HEAD_DIM)
    return w[:, :, _ROPE_PERM].reshape(w.shape[0], n_heads * HEAD_DIM)


def _w_layout(w):
    """[DIM, C] f32 -> [C/512, 2, 128, 16, 512] bf16 slab layout."""
    C = w.shape[1]
    wl = w.reshape(2, KH, P, C // NCH, NCH).transpose(3, 0, 2, 1, 4)
    return np.ascontiguousarray(wl).astype(ml_dtypes.bfloat16)


def _prep_shared(cos, sin, wq, wk, wv, wo):
    wq_p = _permute_heads(np.asarray(wq, dtype=np.float32), N_HEADS)
    wk_p = _permute_heads(np.asarray(wk, dtype=np.float32), N_KV)
    wq_l = _w_layout(wq_p)
    wk_l = _w_layout(wk_p)
    wv_l = _w_layout(np.asarray(wv, dtype=np.float32))
    wo_l = _w_layout(np.asarray(wo, dtype=np.float32))
    # positions restart at 0 per block, so block 0's table serves all cores
    cs_l = np.ascontiguousarray(
        np.asarray(cos[:S], dtype=np.float32).reshape(TT, P, 64).transpose(1, 0, 2)
    )
    sn_l = np.ascontiguousarray(
        np.asarray(sin[:S], dtype=np.float32).reshape(TT, P, 64).transpose(1, 0, 2)
    )
    return cs_l, sn_l, wq_l, wk_l, wv_l, wo_l


def _prep_x_block(xb):
    """x block [S, DIM] f32 -> xt [128, KC, S] bf16 (transposed)."""
    bf = ml_dtypes.bfloat16
    xtb = xb.T.reshape(KC, P, S).transpose(1, 0, 2)
    return np.ascontiguousarray(xtb).astype(bf)


def kernel(x, cos, sin, wq, wk, wv, wo):
    if "nc" not in _CACHE:
        _CACHE["nc"] = build_nc()
    nc = _CACHE["nc"]

    x = np.asarray(x, dtype=np.float32)
    cs_l, sn_l, wq_l, wk_l, wv_l, wo_l = _prep_shared(cos, sin, wq, wk, wv, wo)

    in_maps = []
    for b in range(B):
        in_maps.append({
            "xt": _prep_x_block(x[b * S:(b + 1) * S]),
            "cs": cs_l,
            "sn": sn_l,
            "wq": wq_l,
            "wk": wk_l,
            "wv": wv_l,
            "wo": wo_l,
        })
    _CACHE["last_in_maps"] = in_maps
    res = run_bass_kernel_spmd(nc, in_maps, core_ids=list(range(B)))
    _CACHE["last_results"] = res
    out = np.concatenate([res.results[b]["out"] for b in range(B)], axis=0)
    return out.astype(np.float32)
